# revision 27
# baseline (speedup 1.0000x reference)
"""Trainium2 Bass kernel for single-head full-dim attention (nn_CasualSelfAttention).

Reference math (B=4, S=4096, D=768, fp32):
    q = x @ Wq.T + bq ; k = x @ Wk.T + bk ; v = x @ Wv.T + bv
    att = softmax(q @ k.T * D**-0.5)        # no mask
    y = att @ v
    y = y.transpose(0,2,1).reshape(B,S,D)   # element permutation
    out = y @ Wc.T + bc

Sharding (8 cores): core c = 2*b + h handles batch b, all 4096 queries, its
half of the keys (rows h*2048:(h+1)*2048). Pairwise ReduceScatter (bf16) by
feature rows hands core h the reduced feature slice [384h:384h+384] for all
queries == exactly output rows [2048h:2048h+2048] after the permutation.

v2 numerics / structure:
  - host pre-transposes x and weights (no device transpose DMAs on the way in)
  - exp(z) = 1 + E decomposition: AV matmul runs on the residual E in fp8e4
    (DoubleRow, 2x PE rate) against fp8 v, while the "1"-weighted part is the
    exact bf16 column-sum of v (cv), added in the epilogue. QK^T also runs
    fp8 DoubleRow on fp8 q,k. Projections stay bf16 (precision budget).
  - the value bias bv is applied after normalization (y/s + bv), so v is
    projected without bias and sums need no folding.
  - v's columns are permuted (phi) so the y^T partial rows land in yTaug as
    3 "m-planes" (x = 3u + m), which makes the post-RS permutation scatter a
    set of 4 rectangular 128-partition XBAR transpose DMAs per RS block into
    a sigma-major SBUF buffer fT (col = (s%16)*128 + s//16). Phase F reads fT
    through a 3D weight AP and un-scrambles rows in the output DMA. No DRAM
    roundtrip, no serial transpose tail.
  - sums: DVE accumulates exp tiles pairwise, GpSimd partition_all_reduce
    does the 128->1 key-partition reduction, normalization uses
    broadcast + vector reciprocal.
"""

import numpy as np
import ml_dtypes

BF16 = ml_dtypes.bfloat16
F8 = ml_dtypes.float8_e4m3fn

B, S, D = 4, 4096, 768
SK = S // 2            # keys per core
P = 128
NG = D // P            # 6 feature groups of 128
QC = 512               # query chunk width == RS block width
NQC = S // QC          # 8 query chunks / RS blocks
KT = SK // P           # 16 key tiles
KTP = KT // 2          # 8 key-tile pairs
FH = D // 2            # 384 features per half (RS slice)
SCALE = float(D) ** -0.5
GROUPS = [[0, 1], [2, 3], [4, 5], [6, 7]]

QK_FP8 = True
AV_FP8 = True

_nc = None


def _phi(fp):
    """v-column permutation: ypsum[e] partition p holds feature phi(128e+p)."""
    e, mcol = divmod(fp, P)
    return 384 * (e // 3) + 3 * mcol + (e % 3)


def _scatter_segments():
    """Per (block b, m-plane m): list of (j0, na, gi0, sseg) transpose calls.

    dst[p, a, u] = fn_m[u, j0 + 128a + p] lands at fT3[:, gi0+a, sseg + 16u]
    (natural s columns, stride-16 dst).
    """
    out = {}
    for b in range(NQC):
        c0 = QC * b
        for m in range(3):
            base = m * S + c0
            d0, s0 = base % D, base // D
            segs = []
            jw = D - d0
            if jw >= QC:
                segs.append((0, QC, d0, s0))
            else:
                segs.append((0, jw, d0, s0))
                segs.append((jw, QC - jw, 0, s0 + 1))
            calls = []
            for (j0, jl, dseg, sseg) in segs:
                assert jl % P == 0 and dseg % P == 0 and sseg < 16
                calls.append((j0, jl // P, dseg // P, sseg))
            out[(b, m)] = calls
    return out


def _build_program():
    import concourse.bass as bass
    import concourse.mybir as mybir
    import concourse.tile as tile
    from concourse import bacc
    from concourse import bass_isa

    f32 = mybir.dt.float32
    bf16 = mybir.dt.bfloat16
    fp8 = mybir.dt.float8e4
    Exp = mybir.ActivationFunctionType.Exp
    Identity = mybir.ActivationFunctionType.Identity
    mult = mybir.AluOpType.mult
    DR = mybir.MatmulPerfMode.DoubleRow

    qk_dt = fp8 if QK_FP8 else bf16
    av_dt = fp8 if AV_FP8 else bf16
    segs = _scatter_segments()

    nc = bacc.Bacc(None, num_devices=8)

    xqT = nc.declare_dram_parameter("xqT", [P, NG, S], bf16, isOutput=False)
    xkvT = nc.declare_dram_parameter("xkvT", [P, NG, SK], bf16, isOutput=False)
    wqT = nc.declare_dram_parameter("wqT", [P, NG, D], bf16, isOutput=False)
    wkT = nc.declare_dram_parameter("wkT", [P, NG, D], bf16, isOutput=False)
    wvT = nc.declare_dram_parameter("wvT", [P, NG, D], bf16, isOutput=False)
    wcT = nc.declare_dram_parameter("wcT", [P, NG, D], bf16, isOutput=False)
    bq_c = nc.declare_dram_parameter("bq_c", [P, NG], f32, isOutput=False)
    bk_c = nc.declare_dram_parameter("bk_c", [P, NG], f32, isOutput=False)
    bvout = nc.declare_dram_parameter("bvout", [SK, D], f32, isOutput=False)
    out = nc.declare_dram_parameter("out", [SK, D], f32, isOutput=True)

    with tile.TileContext(nc) as tc:
        with tc.tile_pool(name="persist", bufs=1) as pp, \
             tc.tile_pool(name="dram", bufs=1, space="DRAM") as dram:
            yTaug = [dram.tile([2 * FH, QC], bf16, name=f"yTaug{b}", tag=f"yTaug{b}")
                     for b in range(NQC)]
            rs_out = [dram.tile([FH, QC], bf16, name=f"rs_out{b}", tag=f"rs_out{b}")
                      for b in range(NQC)]
            sums_dr = [dram.tile([1, QC], f32, name=f"sums_dr{b}", tag=f"sums_dr{b}")
                       for b in range(NQC)]
            sums_ar = [dram.tile([1, QC], f32, name=f"sums_ar{b}", tag=f"sums_ar{b}")
                       for b in range(NQC)]

            # ---- persistent SBUF ----
            kT_sb = pp.tile([P, NG, SK], qk_dt, tag="kT")
            qT_sb = pp.tile([P, NG, S], qk_dt, tag="qT")
            v_sb = [pp.tile([P, 2, D], bf16, name=f"v{t}", tag=f"v{t}") for t in range(KTP)]
            v8_sb = [pp.tile([P, 2, D], av_dt, name=f"v8{t}", tag=f"v8{t}") for t in range(KTP)] \
                if AV_FP8 else v_sb
            fT3 = pp.tile([P, NG, SK], bf16, tag="fT3")
            wc_sb = pp.tile([P, NG, D], bf16, tag="wc_sb")
            bq_sb = pp.tile([P, NG], f32, tag="bq_sb")
            bk_sb = pp.tile([P, NG], f32, tag="bk_sb")
            cv_sb = pp.tile([P, NG], f32, tag="cv_sb")
            ones1 = pp.tile([P, 1], bf16, tag="ones1")
            xq0 = pp.tile([P, NG, QC], bf16, tag="xq0")

            wq_sb = pp.tile([P, NG, D], bf16, tag="wq_sb")
            ones8 = pp.tile([P, 8], bf16, tag="ones8")
            neg1 = pp.tile([P, 1], f32, tag="neg1")
            nc.vector.memset(ones1[:], 1.0)
            nc.vector.memset(ones8[:], 1.0)
            nc.vector.memset(neg1[:], -1.0)
            nc.scalar.dma_start(wq_sb[:], wqT[:])
            nc.scalar.dma_start(wc_sb[:], wcT[:])
            nc.scalar.dma_start(xq0[:], xqT[:, :, 0:QC])
            nc.scalar.dma_start(bq_sb[:], bq_c[:])
            nc.scalar.dma_start(bk_sb[:], bk_c[:])

            # ---- Phase A: kT (fp8/bf16), v (bf16 + fp8), cv ----
            with tc.tile_pool(name="pA", bufs=1) as pa, \
                 tc.tile_pool(name="psA", bufs=1, space="PSUM") as psa:
                wk_sb = pa.tile([P, NG, D], bf16, tag="wk_sb")
                wv_sb = pa.tile([P, NG, D], bf16, tag="wv_sb")
                nc.sync.dma_start(wk_sb[:], wkT[:])
                nc.scalar.dma_start(wv_sb[:], wvT[:])
                ones128 = pa.tile([P, P], bf16, tag="ones128")
                nc.vector.memset(ones128[:], 1.0)
                for kc in range(SK // QC):
                    xkv_sb = pa.tile([P, NG, QC], bf16, tag="xkv", bufs=3, name="xkv")
                    nc.sync.dma_start(xkv_sb[:], xkvT[:, :, kc * QC:(kc + 1) * QC])
                    # k projection -> kT_sb
                    for ft in range(NG):
                        ps = psa.tile([P, QC], f32, tag="psk", bufs=2)
                        for g in range(NG):
                            nc.tensor.matmul(ps[:], wk_sb[:, g, ft * P:(ft + 1) * P],
                                             xkv_sb[:, g, :],
                                             start=(g == 0), stop=(g == NG - 1))
                        nc.vector.tensor_scalar_add(
                            kT_sb[:, ft, kc * QC:(kc + 1) * QC], ps[:],
                            bk_sb[:, ft:ft + 1])
                    # v projection (no bias; phi-permuted columns via wvT)
                    for tl in range(QC // P):
                        kt = kc * (QC // P) + tl
                        tpair, jpl = divmod(kt, 2)
                        for half in range(2):
                            ps = psa.tile([P, FH], f32, tag="psv", bufs=2)
                            for g in range(NG):
                                nc.tensor.matmul(
                                    ps[:], xkv_sb[:, g, tl * P:(tl + 1) * P],
                                    wv_sb[:, g, half * FH:(half + 1) * FH],
                                    start=(g == 0), stop=(g == NG - 1))
                            nc.vector.tensor_copy(
                                v_sb[tpair][:, jpl, half * FH:(half + 1) * FH], ps[:])
                            if AV_FP8:
                                nc.vector.tensor_copy(
                                    v8_sb[tpair][:, jpl, half * FH:(half + 1) * FH], ps[:])
                # cv column sums (bf16 v): ones stationary, single accumulation
                # group per PSUM region (columns on the free dim)
                cva = psa.tile([P, QC], f32, tag="cva", bufs=1)
                cvb = psa.tile([P, D - QC], f32, tag="cvb", bufs=1)
                for kt in range(KT):
                    tpair, jpl = divmod(kt, 2)
                    nc.tensor.matmul(cva[:], ones128[:], v_sb[tpair][:, jpl, 0:QC],
                                     start=(kt == 0), stop=(kt == KT - 1))
                    nc.tensor.matmul(cvb[:], ones128[:], v_sb[tpair][:, jpl, QC:D],
                                     start=(kt == 0), stop=(kt == KT - 1))
                cv_row = pa.tile([1, D], f32, tag="cv_row")
                nc.vector.tensor_copy(cv_row[0:1, 0:QC], cva[0:1, :])
                nc.vector.tensor_copy(cv_row[0:1, QC:D], cvb[0:1, :])
                for e in range(NG):
                    nc.gpsimd.dma_start(cv_sb[:, e:e + 1],
                                        cv_row[0:1, e * P:(e + 1) * P])

            # ---- Phase B: qT ----
            with tc.tile_pool(name="pB", bufs=1) as pb, \
                 tc.tile_pool(name="psB", bufs=1, space="PSUM") as psb:
                for c in range(NQC):
                    if c == 0:
                        xq_sb = xq0
                    else:
                        xq_sb = pb.tile([P, NG, QC], bf16, tag="xq", bufs=3, name="xq")
                        nc.sync.dma_start(xq_sb[:], xqT[:, :, c * QC:(c + 1) * QC])
                    for ft in range(NG):
                        ps = psb.tile([P, QC], f32, tag="psq", bufs=2)
                        for g in range(NG):
                            nc.tensor.matmul(ps[:], wq_sb[:, g, ft * P:(ft + 1) * P],
                                             xq_sb[:, g, :],
                                             start=(g == 0), stop=(g == NG - 1))
                        nc.vector.tensor_scalar_add(
                            qT_sb[:, ft, c * QC:(c + 1) * QC], ps[:],
                            bq_sb[:, ft:ft + 1])

            # ---- Phase C: attention + RS + norm + scatter ----
            with tc.tile_pool(name="pC", bufs=2) as pc, \
                 tc.tile_pool(name="pE", bufs=2) as pe, \
                 tc.tile_pool(name="psC", bufs=1, space="PSUM") as psc:

                def qk(kt, qc, aps):
                    if QK_FP8:
                        for a in range(3):
                            nc.tensor.matmul(
                                aps[:], kT_sb[:, 2 * a:2 * a + 2, kt * P:(kt + 1) * P],
                                qT_sb[:, 2 * a:2 * a + 2, qc * QC:(qc + 1) * QC],
                                start=(a == 0), stop=(a == 2), perf_mode=DR)
                    else:
                        for g in range(NG):
                            nc.tensor.matmul(
                                aps[:], kT_sb[:, g, kt * P:(kt + 1) * P],
                                qT_sb[:, g, qc * QC:(qc + 1) * QC],
                                start=(g == 0), stop=(g == NG - 1))

                def av1(t, e_tile, ypsum, e):
                    if AV_FP8:
                        nc.tensor.matmul(
                            ypsum[e][:], v8_sb[t][:, :, e * P:(e + 1) * P],
                            e_tile[:], start=(t == 0), stop=(t == KTP - 1),
                            perf_mode=DR)
                    else:
                        for j in range(2):
                            nc.tensor.matmul(
                                ypsum[e][:], v8_sb[t][:, j, e * P:(e + 1) * P],
                                e_tile[:, j, :],
                                start=(t == 0 and j == 0),
                                stop=(t == KTP - 1 and j == 1))

                def scatter(b):
                    # permutation scatter for RS-completed (already normalized)
                    # block b: rs_out rows -> transposed -> strided fT3 columns
                    fT3r = fT3[:].rearrange("p g (v sg) -> p g v sg", sg=16)
                    for m in range(3):
                        for (j0, na, gi0, sseg) in segs[(b, m)]:
                            tmp = pe.tile([P, 4, P], bf16, tag="scat", bufs=2,
                                          name="scat")
                            nc.sync.dma_start(
                                tmp[:, 0:na, :],
                                rs_out[b][m * P:(m + 1) * P, j0:j0 + na * P],
                                transpose=True)
                            nc.vector.tensor_copy(
                                fT3r[:, gi0:gi0 + na, :, sseg], tmp[:, 0:na, :])

                for qc in range(NQC):
                    ypsum = [psc.tile([P, QC], f32, name=f"y{e}", tag=f"y{e}", bufs=1)
                             for e in range(NG)]
                    sums_ps = psc.tile([8, QC], f32, tag="sums", bufs=1)
                    pairs = {}

                    def sums_mm(kt):
                        t, j = divmod(kt, 2)
                        nc.tensor.matmul(sums_ps[:], ones8[:], pairs[t][0][:, j, :],
                                         start=(kt == 0), stop=(kt == KT - 1),
                                         skip_group_check=True)

                    for t in range(KTP):
                        a_pair = pc.tile([P, 2, QC], bf16, tag="a_pair", bufs=3)
                        e_tile = pc.tile([P, 2, QC], av_dt, tag="e_tile", bufs=3)
                        pairs[t] = (a_pair, e_tile)
                        for j in range(2):
                            kt = 2 * t + j
                            aps = psc.tile([P, QC], f32, tag="att", bufs=1)
                            qk(kt, qc, aps)
                            nc.scalar.activation(a_pair[:, j, :], aps[:], Exp,
                                                 scale=SCALE)
                            nc.scalar.activation(e_tile[:, j, :], a_pair[:, j, :],
                                                 Identity, bias=neg1[:])
                            # fill PE pipeline behind this QK with prev-pair work
                            # so the single aps buffer never stalls the PE
                            if t > 0:
                                sums_mm(2 * (t - 1) + j)
                                for e in (range(3) if j == 0 else range(3, NG)):
                                    av1(t - 1, pairs[t - 1][1], ypsum, e)
                    for j in range(2):
                        sums_mm(2 * (KTP - 1) + j)
                        for e in (range(3) if j == 0 else range(3, NG)):
                            av1(KTP - 1, pairs[KTP - 1][1], ypsum, e)

                    # epilogue: tiny sums AllReduce, normalize, write, RS
                    sums_sb = pc.tile([1, QC], f32, tag="sums_sb")
                    nc.vector.tensor_copy(sums_sb[:], sums_ps[0:1, :])
                    nc.gpsimd.dma_start(sums_dr[qc][:], sums_sb[:])
                    nc.gpsimd.collective_compute(
                        "AllReduce", mybir.AluOpType.add,
                        replica_groups=GROUPS,
                        ins=[sums_dr[qc].opt()], outs=[sums_ar[qc].opt()])
                    s_tot = pc.tile([1, QC], f32, tag="s_tot")
                    nc.gpsimd.dma_start(s_tot[:], sums_ar[qc][:])
                    r32 = pc.tile([1, QC], f32, tag="r32")
                    nc.vector.reciprocal_approx_fast(r32[:], s_tot[:])
                    rec = pc.tile([P, QC], f32, tag="rec")
                    nc.gpsimd.partition_broadcast(rec[:], r32[:])
                    yb = yTaug[qc]
                    stt = mybir.AluOpType.add
                    for e in range(NG):
                        yt = pc.tile([P, QC], f32, tag="yt", bufs=2)
                        nc.vector.tensor_scalar_add(yt[:], ypsum[e][:],
                                                    cv_sb[:, e:e + 1])
                        ytn = pc.tile([P, QC], bf16, tag="ytn", bufs=3)
                        nc.vector.tensor_mul(ytn[:], yt[:], rec[:])
                        half, m = divmod(e, 3)
                        nc.sync.dma_start(
                            yb[FH * half + m * P:FH * half + m * P + P, :],
                            ytn[:])
                    nc.gpsimd.collective_compute(
                        "ReduceScatter", mybir.AluOpType.add,
                        replica_groups=GROUPS,
                        ins=[yTaug[qc].opt()], outs=[rs_out[qc].opt()])
                    if qc > 0:
                        scatter(qc - 1)
                scatter(NQC - 1)

            # ---- Phase F: out = y_perm @ Wc.T + bc ----
            with tc.tile_pool(name="pF", bufs=1) as pf, \
                 tc.tile_pool(name="psF", bufs=2, space="PSUM") as psf:
                for t in range(SK // P):
                    bvt = pf.tile([P, D], f32, tag="bvt", bufs=3, name="bvt")
                    nc.sync.dma_start(bvt[:], bvout[t * P:(t + 1) * P, :])
                    po = psf.tile([P, QC], f32, tag="po")
                    po2 = psf.tile([P, D - QC], f32, tag="po2")
                    for g in range(NG):
                        lhsT = fT3[:, g, t * P:(t + 1) * P]
                        nc.tensor.matmul(po[:], lhsT, wc_sb[:, g, 0:QC],
                                         start=(g == 0), stop=(g == NG - 1))
                        nc.tensor.matmul(po2[:], lhsT, wc_sb[:, g, QC:D],
                                         start=(g == 0), stop=(g == NG - 1))
                    o_sb = pf.tile([P, D], f32, tag="o_sb", bufs=3)
                    nc.vector.tensor_add(o_sb[:, 0:QC], po[:], bvt[:, 0:QC])
                    nc.vector.tensor_add(o_sb[:, QC:D], po2[:], bvt[:, QC:D])
                    nc.sync.dma_start(out[t * P:(t + 1) * P, :], o_sb[:])

    return nc


def _get_nc():
    global _nc
    if _nc is None:
        _nc = _build_program()
        _nc.finalize()
    return _nc


def _gmaj(w):
    # [D, D] (row f_out, col d) -> [P, NG, D]: [p, g, f] = w[f, 128g + p]
    return np.ascontiguousarray(
        w.T.reshape(NG, P, D).transpose(1, 0, 2)).astype(BF16)


def _prep_in_maps(x, Wq, bq, Wk, bk, Wv, bv, Wc, bc):
    x = np.asarray(x, dtype=np.float32)
    Wq = np.asarray(Wq, np.float32); Wk = np.asarray(Wk, np.float32)
    Wv = np.asarray(Wv, np.float32); Wc = np.asarray(Wc, np.float32)
    bqf = np.asarray(bq, np.float32); bkf = np.asarray(bk, np.float32)
    bvf = np.asarray(bv, np.float32); bcf = np.asarray(bc, np.float32)

    phi = np.array([_phi(f) for f in range(D)])
    wq4 = _gmaj(Wq)
    wk4 = _gmaj(Wk)
    wv4 = _gmaj(Wv[phi])          # permuted output columns
    wc4 = _gmaj(Wc)               # wc_sb[p, g, e] = Wc[e, 128g+p]
    bq_c = np.ascontiguousarray(bqf.reshape(NG, P).T)
    bk_c = np.ascontiguousarray(bkf.reshape(NG, P).T)

    # bvout[s, e] = sum_d bv[f(s, d)] * Wc[e, d] + bc[e], per half h:
    # the +bv term of the normalized y, pushed through the permutation and
    # the output projection on the host.
    bvouts = []
    for h in range(2):
        flat = 768 * (SK * h) + np.arange(SK * D)
        ybv = bvf[flat // S].reshape(SK, D)      # y_perm rows of the bv field
        bvouts.append((ybv @ Wc.T + bcf).astype(np.float32))

    in_maps = []
    for c in range(8):
        b, h = divmod(c, 2)
        xT = x[b].T.astype(BF16)                      # [D, S]
        xq4 = np.ascontiguousarray(xT.reshape(NG, P, S).transpose(1, 0, 2))
        xkv4 = np.ascontiguousarray(
            xT[:, h * SK:(h + 1) * SK].reshape(NG, P, SK).transpose(1, 0, 2))
        in_maps.append({
            "xqT": xq4, "xkvT": xkv4,
            "wqT": wq4, "wkT": wk4, "wvT": wv4, "wcT": wc4,
            "bq_c": bq_c, "bk_c": bk_c, "bvout": bvouts[h],
        })
    return in_maps


def _assemble(results):
    out = np.empty((B, S, D), dtype=np.float32)
    for c in range(8):
        b, h = divmod(c, 2)
        out[b, h * SK:(h + 1) * SK, :] = results[c]["out"]
    return out


def run_on_hw(trace=False, **inputs):
    from concourse.bass_utils import run_bass_kernel_spmd
    nc = _get_nc()
    in_maps = _prep_in_maps(**inputs)
    res = run_bass_kernel_spmd(nc, in_maps, list(range(8)), trace=trace)
    return _assemble(res.results), res


def kernel(**inputs):
    out, _ = run_on_hw(trace=False, **inputs)
    return out


# revision 28
# speedup vs baseline: 1.0787x; 1.0787x over previous
"""Trainium2 Bass kernel for single-head full-dim attention (nn_CasualSelfAttention).

Reference math (B=4, S=4096, D=768, fp32):
    q = x @ Wq.T + bq ; k = x @ Wk.T + bk ; v = x @ Wv.T + bv
    att = softmax(q @ k.T * D**-0.5)        # no mask
    y = att @ v
    y = y.transpose(0,2,1).reshape(B,S,D)   # element permutation
    out = y @ Wc.T + bc

Sharding (8 cores): core c = 2*b + h handles batch b, all 4096 queries, its
half of the keys (rows h*2048:(h+1)*2048). Pairwise ReduceScatter (bf16) by
feature rows hands core h the reduced feature slice [384h:384h+384] for all
queries == exactly output rows [2048h:2048h+2048] after the permutation.

v2 numerics / structure:
  - host pre-transposes x and weights (no device transpose DMAs on the way in)
  - exp(z) = 1 + E decomposition: AV matmul runs on the residual E in fp8e4
    (DoubleRow, 2x PE rate) against fp8 v, while the "1"-weighted part is the
    exact bf16 column-sum of v (cv), added in the epilogue. QK^T also runs
    fp8 DoubleRow on fp8 q,k. Projections stay bf16 (precision budget).
  - the value bias bv is applied after normalization (y/s + bv), so v is
    projected without bias and sums need no folding.
  - v's columns are permuted (phi) so the y^T partial rows land in yTaug as
    3 "m-planes" (x = 3u + m), which makes the post-RS permutation scatter a
    set of 4 rectangular 128-partition XBAR transpose DMAs per RS block into
    a sigma-major SBUF buffer fT (col = (s%16)*128 + s//16). Phase F reads fT
    through a 3D weight AP and un-scrambles rows in the output DMA. No DRAM
    roundtrip, no serial transpose tail.
  - sums: DVE accumulates exp tiles pairwise, GpSimd partition_all_reduce
    does the 128->1 key-partition reduction, normalization uses
    broadcast + vector reciprocal.
"""

import numpy as np
import ml_dtypes

BF16 = ml_dtypes.bfloat16
F8 = ml_dtypes.float8_e4m3fn

B, S, D = 4, 4096, 768
SK = S // 2            # keys per core
P = 128
NG = D // P            # 6 feature groups of 128
QC = 512               # query chunk width == RS block width
NQC = S // QC          # 8 query chunks / RS blocks
KT = SK // P           # 16 key tiles
KTP = KT // 2          # 8 key-tile pairs
FH = D // 2            # 384 features per half (RS slice)
SCALE = float(D) ** -0.5
GROUPS = [[0, 1], [2, 3], [4, 5], [6, 7]]

QK_FP8 = True
AV_FP8 = True

_nc = None


def _phi(fp):
    """v-column permutation: ypsum[e] partition p holds feature phi(128e+p)."""
    e, mcol = divmod(fp, P)
    return 384 * (e // 3) + 3 * mcol + (e % 3)


def _scatter_segments():
    """Per (block b, m-plane m): list of (j0, na, gi0, sseg) transpose calls.

    dst[p, a, u] = fn_m[u, j0 + 128a + p] lands at fT3[:, gi0+a, sseg + 16u]
    (natural s columns, stride-16 dst).
    """
    out = {}
    for b in range(NQC):
        c0 = QC * b
        for m in range(3):
            base = m * S + c0
            d0, s0 = base % D, base // D
            segs = []
            jw = D - d0
            if jw >= QC:
                segs.append((0, QC, d0, s0))
            else:
                segs.append((0, jw, d0, s0))
                segs.append((jw, QC - jw, 0, s0 + 1))
            calls = []
            for (j0, jl, dseg, sseg) in segs:
                assert jl % P == 0 and dseg % P == 0 and sseg < 16
                calls.append((j0, jl // P, dseg // P, sseg))
            out[(b, m)] = calls
    return out


def _build_program():
    import concourse.bass as bass
    import concourse.mybir as mybir
    import concourse.tile as tile
    from concourse import bacc
    from concourse import bass_isa

    f32 = mybir.dt.float32
    bf16 = mybir.dt.bfloat16
    fp8 = mybir.dt.float8e4
    Exp = mybir.ActivationFunctionType.Exp
    Identity = mybir.ActivationFunctionType.Identity
    mult = mybir.AluOpType.mult
    DR = mybir.MatmulPerfMode.DoubleRow

    qk_dt = fp8 if QK_FP8 else bf16
    av_dt = fp8 if AV_FP8 else bf16
    segs = _scatter_segments()

    nc = bacc.Bacc(None, num_devices=8)

    xqT = nc.declare_dram_parameter("xqT", [P, NG, S], bf16, isOutput=False)
    xkvT = nc.declare_dram_parameter("xkvT", [P, NG, SK], bf16, isOutput=False)
    wqT = nc.declare_dram_parameter("wqT", [P, NG, D], bf16, isOutput=False)
    wkT = nc.declare_dram_parameter("wkT", [P, NG, D], bf16, isOutput=False)
    wvT = nc.declare_dram_parameter("wvT", [P, NG, D], bf16, isOutput=False)
    wcT = nc.declare_dram_parameter("wcT", [P, NG, D], bf16, isOutput=False)
    bq_c = nc.declare_dram_parameter("bq_c", [P, NG], f32, isOutput=False)
    bk_c = nc.declare_dram_parameter("bk_c", [P, NG], f32, isOutput=False)
    bvout = nc.declare_dram_parameter("bvout", [SK, D], f32, isOutput=False)
    out = nc.declare_dram_parameter("out", [SK, D], f32, isOutput=True)

    with tile.TileContext(nc) as tc:
        with tc.tile_pool(name="persist", bufs=1) as pp, \
             tc.tile_pool(name="dram", bufs=1, space="DRAM") as dram:
            yTaug = [dram.tile([2 * FH, QC], bf16, name=f"yTaug{b}", tag=f"yTaug{b}")
                     for b in range(NQC)]
            rs_out = [dram.tile([FH, QC], bf16, name=f"rs_out{b}", tag=f"rs_out{b}")
                      for b in range(NQC)]
            sums_dr = [dram.tile([1, QC], f32, name=f"sums_dr{b}", tag=f"sums_dr{b}")
                       for b in range(NQC)]
            sums_ar = [dram.tile([1, QC], f32, name=f"sums_ar{b}", tag=f"sums_ar{b}")
                       for b in range(NQC)]

            # ---- persistent SBUF ----
            kT_sb = pp.tile([P, NG, SK], qk_dt, tag="kT")
            qT_sb = pp.tile([P, NG, S], qk_dt, tag="qT")
            v_sb = [pp.tile([P, 2, D], bf16, name=f"v{t}", tag=f"v{t}") for t in range(KTP)]
            v8_sb = [pp.tile([P, 2, D], av_dt, name=f"v8{t}", tag=f"v8{t}") for t in range(KTP)] \
                if AV_FP8 else v_sb
            fT3 = pp.tile([P, NG, SK], bf16, tag="fT3")
            wc_sb = pp.tile([P, NG, D], bf16, tag="wc_sb")
            bq_sb = pp.tile([P, NG], f32, tag="bq_sb")
            bk_sb = pp.tile([P, NG], f32, tag="bk_sb")
            cv_sb = pp.tile([P, NG], f32, tag="cv_sb")
            ones1 = pp.tile([P, 1], bf16, tag="ones1")
            xq0 = pp.tile([P, NG, QC], bf16, tag="xq0")

            wq_sb = pp.tile([P, NG, D], bf16, tag="wq_sb")
            ones8 = pp.tile([P, 8], bf16, tag="ones8")
            neg1 = pp.tile([P, 1], f32, tag="neg1")
            nc.vector.memset(ones1[:], 1.0)
            nc.vector.memset(ones8[:], 1.0)
            nc.vector.memset(neg1[:], -1.0)
            nc.scalar.dma_start(bq_sb[:], bq_c[:])
            nc.scalar.dma_start(bk_sb[:], bk_c[:])
            nc.scalar.dma_start(wq_sb[:], wqT[:])
            nc.scalar.dma_start(xq0[:], xqT[:, :, 0:QC])
            nc.scalar.dma_start(wc_sb[:], wcT[:])

            # ---- Phase A: kT (fp8/bf16), v (bf16 + fp8), cv ----
            with tc.tile_pool(name="pA", bufs=1) as pa, \
                 tc.tile_pool(name="psA", bufs=1, space="PSUM") as psa:
                wk_sb = pa.tile([P, NG, D], bf16, tag="wk_sb")
                wv_sb = pa.tile([P, NG, D], bf16, tag="wv_sb")
                nc.sync.dma_start(wk_sb[:], wkT[:])
                nc.sync.dma_start(wv_sb[:], wvT[:])
                ones128 = pa.tile([P, P], bf16, tag="ones128")
                nc.vector.memset(ones128[:], 1.0)
                for kc in range(SK // QC):
                    xkv_sb = pa.tile([P, NG, QC], bf16, tag="xkv", bufs=3, name="xkv")
                    nc.sync.dma_start(xkv_sb[:], xkvT[:, :, kc * QC:(kc + 1) * QC])
                    # k projection -> kT_sb
                    for ft in range(NG):
                        ps = psa.tile([P, QC], f32, tag="psk", bufs=2)
                        for g in range(NG):
                            nc.tensor.matmul(ps[:], wk_sb[:, g, ft * P:(ft + 1) * P],
                                             xkv_sb[:, g, :],
                                             start=(g == 0), stop=(g == NG - 1))
                        nc.vector.tensor_scalar_add(
                            kT_sb[:, ft, kc * QC:(kc + 1) * QC], ps[:],
                            bk_sb[:, ft:ft + 1])
                    # v projection (no bias; phi-permuted columns via wvT)
                    for tl in range(QC // P):
                        kt = kc * (QC // P) + tl
                        tpair, jpl = divmod(kt, 2)
                        for half in range(2):
                            ps = psa.tile([P, FH], f32, tag="psv", bufs=2)
                            for g in range(NG):
                                nc.tensor.matmul(
                                    ps[:], xkv_sb[:, g, tl * P:(tl + 1) * P],
                                    wv_sb[:, g, half * FH:(half + 1) * FH],
                                    start=(g == 0), stop=(g == NG - 1))
                            nc.vector.tensor_copy(
                                v_sb[tpair][:, jpl, half * FH:(half + 1) * FH], ps[:])
                            if AV_FP8:
                                nc.vector.tensor_copy(
                                    v8_sb[tpair][:, jpl, half * FH:(half + 1) * FH], ps[:])
                # cv column sums (bf16 v): ones stationary, single accumulation
                # group per PSUM region (columns on the free dim)
                cva = psa.tile([P, QC], f32, tag="cva", bufs=1)
                cvb = psa.tile([P, D - QC], f32, tag="cvb", bufs=1)
                for kt in range(KT):
                    tpair, jpl = divmod(kt, 2)
                    nc.tensor.matmul(cva[:], ones128[:], v_sb[tpair][:, jpl, 0:QC],
                                     start=(kt == 0), stop=(kt == KT - 1))
                    nc.tensor.matmul(cvb[:], ones128[:], v_sb[tpair][:, jpl, QC:D],
                                     start=(kt == 0), stop=(kt == KT - 1))
                cv_row = pa.tile([1, D], f32, tag="cv_row")
                nc.vector.tensor_copy(cv_row[0:1, 0:QC], cva[0:1, :])
                nc.vector.tensor_copy(cv_row[0:1, QC:D], cvb[0:1, :])
                for e in range(NG):
                    nc.gpsimd.dma_start(cv_sb[:, e:e + 1],
                                        cv_row[0:1, e * P:(e + 1) * P])

            # ---- Phase B: qT ----
            with tc.tile_pool(name="pB", bufs=1) as pb, \
                 tc.tile_pool(name="psB", bufs=1, space="PSUM") as psb:
                for c in range(NQC):
                    if c == 0:
                        xq_sb = xq0
                    else:
                        xq_sb = pb.tile([P, NG, QC], bf16, tag="xq", bufs=3, name="xq")
                        nc.sync.dma_start(xq_sb[:], xqT[:, :, c * QC:(c + 1) * QC])
                    for ft in range(NG):
                        ps = psb.tile([P, QC], f32, tag="psq", bufs=2)
                        for g in range(NG):
                            nc.tensor.matmul(ps[:], wq_sb[:, g, ft * P:(ft + 1) * P],
                                             xq_sb[:, g, :],
                                             start=(g == 0), stop=(g == NG - 1))
                        nc.vector.tensor_scalar_add(
                            qT_sb[:, ft, c * QC:(c + 1) * QC], ps[:],
                            bq_sb[:, ft:ft + 1])

            # ---- Phase C: attention + RS + norm + scatter ----
            with tc.tile_pool(name="pC", bufs=2) as pc, \
                 tc.tile_pool(name="pE", bufs=2) as pe, \
                 tc.tile_pool(name="psC", bufs=1, space="PSUM") as psc:

                def qk(kt, qc, aps):
                    if QK_FP8:
                        for a in range(3):
                            nc.tensor.matmul(
                                aps[:], kT_sb[:, 2 * a:2 * a + 2, kt * P:(kt + 1) * P],
                                qT_sb[:, 2 * a:2 * a + 2, qc * QC:(qc + 1) * QC],
                                start=(a == 0), stop=(a == 2), perf_mode=DR)
                    else:
                        for g in range(NG):
                            nc.tensor.matmul(
                                aps[:], kT_sb[:, g, kt * P:(kt + 1) * P],
                                qT_sb[:, g, qc * QC:(qc + 1) * QC],
                                start=(g == 0), stop=(g == NG - 1))

                def av1(t, e_tile, ypsum, e):
                    if AV_FP8:
                        nc.tensor.matmul(
                            ypsum[e][:], v8_sb[t][:, :, e * P:(e + 1) * P],
                            e_tile[:], start=(t == 0), stop=(t == KTP - 1),
                            perf_mode=DR)
                    else:
                        for j in range(2):
                            nc.tensor.matmul(
                                ypsum[e][:], v8_sb[t][:, j, e * P:(e + 1) * P],
                                e_tile[:, j, :],
                                start=(t == 0 and j == 0),
                                stop=(t == KTP - 1 and j == 1))

                def scatter(b):
                    # permutation scatter for RS-completed (already normalized)
                    # block b: rs_out rows -> transposed -> strided fT3 columns
                    fT3r = fT3[:].rearrange("p g (v sg) -> p g v sg", sg=16)
                    for m in range(3):
                        for (j0, na, gi0, sseg) in segs[(b, m)]:
                            tmp = pe.tile([P, 4, P], bf16, tag="scat", bufs=2,
                                          name="scat")
                            nc.sync.dma_start(
                                tmp[:, 0:na, :],
                                rs_out[b][m * P:(m + 1) * P, j0:j0 + na * P],
                                transpose=True)
                            nc.vector.tensor_copy(
                                fT3r[:, gi0:gi0 + na, :, sseg], tmp[:, 0:na, :])

                for qc in range(NQC):
                    ypsum = [psc.tile([P, QC], f32, name=f"y{e}", tag=f"y{e}", bufs=1)
                             for e in range(NG)]
                    sums_ps = psc.tile([8, QC], f32, tag="sums", bufs=1)
                    pairs = {}

                    def sums_mm(kt):
                        t, j = divmod(kt, 2)
                        nc.tensor.matmul(sums_ps[:], ones8[:], pairs[t][0][:, j, :],
                                         start=(kt == 0), stop=(kt == KT - 1),
                                         skip_group_check=True)

                    for t in range(KTP):
                        a_pair = pc.tile([P, 2, QC], bf16, tag="a_pair", bufs=3)
                        e_tile = pc.tile([P, 2, QC], av_dt, tag="e_tile", bufs=3)
                        pairs[t] = (a_pair, e_tile)
                        for j in range(2):
                            kt = 2 * t + j
                            aps = psc.tile([P, QC], f32, tag="att", bufs=1)
                            qk(kt, qc, aps)
                            nc.scalar.activation(a_pair[:, j, :], aps[:], Exp,
                                                 scale=SCALE)
                            nc.scalar.activation(e_tile[:, j, :], a_pair[:, j, :],
                                                 Identity, bias=neg1[:])
                            # fill PE pipeline behind this QK with prev-pair work
                            # so the single aps buffer never stalls the PE
                            if t > 0:
                                sums_mm(2 * (t - 1) + j)
                                for e in (range(3) if j == 0 else range(3, NG)):
                                    av1(t - 1, pairs[t - 1][1], ypsum, e)
                    for j in range(2):
                        sums_mm(2 * (KTP - 1) + j)
                        for e in (range(3) if j == 0 else range(3, NG)):
                            av1(KTP - 1, pairs[KTP - 1][1], ypsum, e)

                    # epilogue: tiny sums AllReduce, normalize, write, RS
                    sums_sb = pc.tile([1, QC], f32, tag="sums_sb")
                    nc.vector.tensor_copy(sums_sb[:], sums_ps[0:1, :])
                    nc.gpsimd.dma_start(sums_dr[qc][:], sums_sb[:])
                    nc.gpsimd.collective_compute(
                        "AllReduce", mybir.AluOpType.add,
                        replica_groups=GROUPS,
                        ins=[sums_dr[qc].opt()], outs=[sums_ar[qc].opt()])
                    s_tot = pc.tile([1, QC], f32, tag="s_tot")
                    nc.gpsimd.dma_start(s_tot[:], sums_ar[qc][:])
                    r32 = pc.tile([1, QC], f32, tag="r32")
                    nc.vector.reciprocal_approx_fast(r32[:], s_tot[:])
                    rec = pc.tile([P, QC], f32, tag="rec")
                    nc.gpsimd.partition_broadcast(rec[:], r32[:])
                    yb = yTaug[qc]
                    yts = []
                    for e in range(NG):
                        yt = pc.tile([P, QC], f32, tag=f"yt{e}", bufs=1)
                        nc.vector.tensor_scalar_add(yt[:], ypsum[e][:],
                                                    cv_sb[:, e:e + 1])
                        yts.append(yt)
                    for e in range(NG):
                        ytn = pc.tile([P, QC], bf16, tag="ytn", bufs=3)
                        nc.vector.tensor_mul(ytn[:], yts[e][:], rec[:])
                        half, m = divmod(e, 3)
                        nc.sync.dma_start(
                            yb[FH * half + m * P:FH * half + m * P + P, :],
                            ytn[:])
                    nc.gpsimd.collective_compute(
                        "ReduceScatter", mybir.AluOpType.add,
                        replica_groups=GROUPS,
                        ins=[yTaug[qc].opt()], outs=[rs_out[qc].opt()])
                    if qc > 0:
                        scatter(qc - 1)
                scatter(NQC - 1)

            # ---- Phase F: out = y_perm @ Wc.T + bc ----
            with tc.tile_pool(name="pF", bufs=1) as pf, \
                 tc.tile_pool(name="psF", bufs=2, space="PSUM") as psf:
                for t in range(SK // P):
                    bvt = pf.tile([P, D], f32, tag="bvt", bufs=3, name="bvt")
                    nc.scalar.dma_start(bvt[:], bvout[t * P:(t + 1) * P, :])
                    po = psf.tile([P, QC], f32, tag="po")
                    po2 = psf.tile([P, D - QC], f32, tag="po2")
                    for g in range(NG):
                        lhsT = fT3[:, g, t * P:(t + 1) * P]
                        nc.tensor.matmul(po[:], lhsT, wc_sb[:, g, 0:QC],
                                         start=(g == 0), stop=(g == NG - 1))
                        nc.tensor.matmul(po2[:], lhsT, wc_sb[:, g, QC:D],
                                         start=(g == 0), stop=(g == NG - 1))
                    o_sb = pf.tile([P, D], f32, tag="o_sb", bufs=3)
                    nc.vector.tensor_add(o_sb[:, 0:QC], po[:], bvt[:, 0:QC])
                    nc.vector.tensor_add(o_sb[:, QC:D], po2[:], bvt[:, QC:D])
                    nc.sync.dma_start(out[t * P:(t + 1) * P, :], o_sb[:])

    return nc


def _get_nc():
    global _nc
    if _nc is None:
        _nc = _build_program()
        _nc.finalize()
    return _nc


def _gmaj(w):
    # [D, D] (row f_out, col d) -> [P, NG, D]: [p, g, f] = w[f, 128g + p]
    return np.ascontiguousarray(
        w.T.reshape(NG, P, D).transpose(1, 0, 2)).astype(BF16)


def _prep_in_maps(x, Wq, bq, Wk, bk, Wv, bv, Wc, bc):
    x = np.asarray(x, dtype=np.float32)
    Wq = np.asarray(Wq, np.float32); Wk = np.asarray(Wk, np.float32)
    Wv = np.asarray(Wv, np.float32); Wc = np.asarray(Wc, np.float32)
    bqf = np.asarray(bq, np.float32); bkf = np.asarray(bk, np.float32)
    bvf = np.asarray(bv, np.float32); bcf = np.asarray(bc, np.float32)

    phi = np.array([_phi(f) for f in range(D)])
    wq4 = _gmaj(Wq)
    wk4 = _gmaj(Wk)
    wv4 = _gmaj(Wv[phi])          # permuted output columns
    wc4 = _gmaj(Wc)               # wc_sb[p, g, e] = Wc[e, 128g+p]
    bq_c = np.ascontiguousarray(bqf.reshape(NG, P).T)
    bk_c = np.ascontiguousarray(bkf.reshape(NG, P).T)

    # bvout[s, e] = sum_d bv[f(s, d)] * Wc[e, d] + bc[e], per half h:
    # the +bv term of the normalized y, pushed through the permutation and
    # the output projection on the host.
    bvouts = []
    for h in range(2):
        flat = 768 * (SK * h) + np.arange(SK * D)
        ybv = bvf[flat // S].reshape(SK, D)      # y_perm rows of the bv field
        bvouts.append((ybv @ Wc.T + bcf).astype(np.float32))

    in_maps = []
    for c in range(8):
        b, h = divmod(c, 2)
        xT = x[b].T.astype(BF16)                      # [D, S]
        xq4 = np.ascontiguousarray(xT.reshape(NG, P, S).transpose(1, 0, 2))
        xkv4 = np.ascontiguousarray(
            xT[:, h * SK:(h + 1) * SK].reshape(NG, P, SK).transpose(1, 0, 2))
        in_maps.append({
            "xqT": xq4, "xkvT": xkv4,
            "wqT": wq4, "wkT": wk4, "wvT": wv4, "wcT": wc4,
            "bq_c": bq_c, "bk_c": bk_c, "bvout": bvouts[h],
        })
    return in_maps


def _assemble(results):
    out = np.empty((B, S, D), dtype=np.float32)
    for c in range(8):
        b, h = divmod(c, 2)
        out[b, h * SK:(h + 1) * SK, :] = results[c]["out"]
    return out


def run_on_hw(trace=False, **inputs):
    from concourse.bass_utils import run_bass_kernel_spmd
    nc = _get_nc()
    in_maps = _prep_in_maps(**inputs)
    res = run_bass_kernel_spmd(nc, in_maps, list(range(8)), trace=trace)
    return _assemble(res.results), res


def kernel(**inputs):
    out, _ = run_on_hw(trace=False, **inputs)
    return out


# revision 29
# speedup vs baseline: 1.1719x; 1.0863x over previous
"""Trainium2 Bass kernel for single-head full-dim attention (nn_CasualSelfAttention).

Reference math (B=4, S=4096, D=768, fp32):
    q = x @ Wq.T + bq ; k = x @ Wk.T + bk ; v = x @ Wv.T + bv
    att = softmax(q @ k.T * D**-0.5)        # no mask
    y = att @ v
    y = y.transpose(0,2,1).reshape(B,S,D)   # element permutation
    out = y @ Wc.T + bc

Sharding (8 cores): core c = 2*b + h handles batch b, all 4096 queries, its
half of the keys (rows h*2048:(h+1)*2048). Pairwise ReduceScatter (bf16) by
feature rows hands core h the reduced feature slice [384h:384h+384] for all
queries == exactly output rows [2048h:2048h+2048] after the permutation.

v2 numerics / structure:
  - host pre-transposes x and weights (no device transpose DMAs on the way in)
  - exp(z) = 1 + E decomposition: AV matmul runs on the residual E in fp8e4
    (DoubleRow, 2x PE rate) against fp8 v, while the "1"-weighted part is the
    exact bf16 column-sum of v (cv), added in the epilogue. QK^T also runs
    fp8 DoubleRow on fp8 q,k. Projections stay bf16 (precision budget).
  - the value bias bv is applied after normalization (y/s + bv), so v is
    projected without bias and sums need no folding.
  - v's columns are permuted (phi) so the y^T partial rows land in yTaug as
    3 "m-planes" (x = 3u + m), which makes the post-RS permutation scatter a
    set of 4 rectangular 128-partition XBAR transpose DMAs per RS block into
    a sigma-major SBUF buffer fT (col = (s%16)*128 + s//16). Phase F reads fT
    through a 3D weight AP and un-scrambles rows in the output DMA. No DRAM
    roundtrip, no serial transpose tail.
  - sums: DVE accumulates exp tiles pairwise, GpSimd partition_all_reduce
    does the 128->1 key-partition reduction, normalization uses
    broadcast + vector reciprocal.
"""

import numpy as np
import ml_dtypes

BF16 = ml_dtypes.bfloat16
F8 = ml_dtypes.float8_e4m3fn

B, S, D = 4, 4096, 768
SK = S // 2            # keys per core
P = 128
NG = D // P            # 6 feature groups of 128
QC = 512               # query chunk width == RS block width
NQC = S // QC          # 8 query chunks / RS blocks
KT = SK // P           # 16 key tiles
KTP = KT // 2          # 8 key-tile pairs
FH = D // 2            # 384 features per half (RS slice)
SCALE = float(D) ** -0.5
GROUPS = [[0, 1], [2, 3], [4, 5], [6, 7]]

QK_FP8 = True
AV_FP8 = True

_nc = None


def _phi(fp):
    """v-column permutation: ypsum[e] partition p holds feature phi(128e+p)."""
    e, mcol = divmod(fp, P)
    return 384 * (e // 3) + 3 * mcol + (e % 3)


def _scatter_segments():
    """Per (block b, m-plane m): list of (j0, na, gi0, sseg) transpose calls.

    dst[p, a, u] = fn_m[u, j0 + 128a + p] lands at fT3[:, gi0+a, sseg + 16u]
    (natural s columns, stride-16 dst).
    """
    out = {}
    for b in range(NQC):
        c0 = QC * b
        for m in range(3):
            base = m * S + c0
            d0, s0 = base % D, base // D
            segs = []
            jw = D - d0
            if jw >= QC:
                segs.append((0, QC, d0, s0))
            else:
                segs.append((0, jw, d0, s0))
                segs.append((jw, QC - jw, 0, s0 + 1))
            calls = []
            for (j0, jl, dseg, sseg) in segs:
                assert jl % P == 0 and dseg % P == 0 and sseg < 16
                calls.append((j0, jl // P, dseg // P, sseg))
            out[(b, m)] = calls
    return out


def _build_program():
    import concourse.bass as bass
    import concourse.mybir as mybir
    import concourse.tile as tile
    from concourse import bacc
    from concourse import bass_isa

    f32 = mybir.dt.float32
    bf16 = mybir.dt.bfloat16
    fp8 = mybir.dt.float8e4
    Exp = mybir.ActivationFunctionType.Exp
    Identity = mybir.ActivationFunctionType.Identity
    mult = mybir.AluOpType.mult
    DR = mybir.MatmulPerfMode.DoubleRow

    qk_dt = fp8 if QK_FP8 else bf16
    av_dt = fp8 if AV_FP8 else bf16
    segs = _scatter_segments()

    nc = bacc.Bacc(None, num_devices=8)

    xqT = nc.declare_dram_parameter("xqT", [P, NG, S], bf16, isOutput=False)
    xkvT = nc.declare_dram_parameter("xkvT", [P, NG, SK], bf16, isOutput=False)
    wqT = nc.declare_dram_parameter("wqT", [P, NG, D], bf16, isOutput=False)
    wkT = nc.declare_dram_parameter("wkT", [P, NG, D], bf16, isOutput=False)
    wvT = nc.declare_dram_parameter("wvT", [P, NG, D], bf16, isOutput=False)
    wcT = nc.declare_dram_parameter("wcT", [P, NG, D], bf16, isOutput=False)
    bq_c = nc.declare_dram_parameter("bq_c", [P, NG], f32, isOutput=False)
    bk_c = nc.declare_dram_parameter("bk_c", [P, NG], f32, isOutput=False)
    bvout = nc.declare_dram_parameter("bvout", [SK, D], f32, isOutput=False)
    out = nc.declare_dram_parameter("out", [SK, D], f32, isOutput=True)

    with tile.TileContext(nc) as tc:
        with tc.tile_pool(name="persist", bufs=1) as pp, \
             tc.tile_pool(name="dram", bufs=1, space="DRAM") as dram:
            yTaug = [dram.tile([2 * (FH + 1), QC], bf16, name=f"yTaug{b}", tag=f"yTaug{b}")
                     for b in range(NQC)]
            rs_out = [dram.tile([FH + 1, QC], bf16, name=f"rs_out{b}", tag=f"rs_out{b}")
                      for b in range(NQC)]

            # ---- persistent SBUF ----
            kT_sb = pp.tile([P, NG, SK], qk_dt, tag="kT")
            qT_sb = pp.tile([P, NG, S], qk_dt, tag="qT")
            v_sb = [pp.tile([P, 2, D], bf16, name=f"v{t}", tag=f"v{t}") for t in range(KTP)]
            v8_sb = [pp.tile([P, 2, D], av_dt, name=f"v8{t}", tag=f"v8{t}") for t in range(KTP)] \
                if AV_FP8 else v_sb
            fT3 = pp.tile([P, NG, SK], bf16, tag="fT3")
            wc_sb = pp.tile([P, NG, D], bf16, tag="wc_sb")
            bq_sb = pp.tile([P, NG], f32, tag="bq_sb")
            bk_sb = pp.tile([P, NG], f32, tag="bk_sb")
            cv_sb = pp.tile([P, NG], f32, tag="cv_sb")
            ones1 = pp.tile([P, 1], bf16, tag="ones1")
            xq0 = pp.tile([P, NG, QC], bf16, tag="xq0")

            wq_sb = pp.tile([P, NG, D], bf16, tag="wq_sb")
            ones8 = pp.tile([P, 8], bf16, tag="ones8")
            neg1 = pp.tile([P, 1], f32, tag="neg1")
            nc.vector.memset(ones1[:], 1.0)
            nc.vector.memset(ones8[:], 1.0)
            nc.vector.memset(neg1[:], -1.0)
            nc.scalar.dma_start(bq_sb[:], bq_c[:])
            nc.scalar.dma_start(bk_sb[:], bk_c[:])
            nc.scalar.dma_start(wq_sb[:], wqT[:])
            nc.scalar.dma_start(xq0[:], xqT[:, :, 0:QC])
            nc.scalar.dma_start(wc_sb[:], wcT[:])

            # ---- Phase A: kT (fp8/bf16), v (bf16 + fp8), cv ----
            with tc.tile_pool(name="pA", bufs=1) as pa, \
                 tc.tile_pool(name="psA", bufs=1, space="PSUM") as psa:
                wk_sb = pa.tile([P, NG, D], bf16, tag="wk_sb")
                wv_sb = pa.tile([P, NG, D], bf16, tag="wv_sb")
                nc.sync.dma_start(wk_sb[:], wkT[:])
                ones128 = pa.tile([P, P], bf16, tag="ones128")
                nc.vector.memset(ones128[:], 1.0)
                wv_loaded = False
                for kc in range(SK // QC):
                    xkv_sb = pa.tile([P, NG, QC], bf16, tag="xkv", bufs=3, name="xkv")
                    nc.sync.dma_start(xkv_sb[:], xkvT[:, :, kc * QC:(kc + 1) * QC])
                    if not wv_loaded:
                        nc.sync.dma_start(wv_sb[:], wvT[:])
                        wv_loaded = True
                    # k projection -> kT_sb
                    for ft in range(NG):
                        ps = psa.tile([P, QC], f32, tag="psk", bufs=2)
                        for g in range(NG):
                            nc.tensor.matmul(ps[:], wk_sb[:, g, ft * P:(ft + 1) * P],
                                             xkv_sb[:, g, :],
                                             start=(g == 0), stop=(g == NG - 1))
                        nc.vector.tensor_scalar_add(
                            kT_sb[:, ft, kc * QC:(kc + 1) * QC], ps[:],
                            bk_sb[:, ft:ft + 1])
                    # v projection (no bias; phi-permuted columns via wvT)
                    for tl in range(QC // P):
                        kt = kc * (QC // P) + tl
                        tpair, jpl = divmod(kt, 2)
                        for half in range(2):
                            ps = psa.tile([P, FH], f32, tag="psv", bufs=2)
                            for g in range(NG):
                                nc.tensor.matmul(
                                    ps[:], xkv_sb[:, g, tl * P:(tl + 1) * P],
                                    wv_sb[:, g, half * FH:(half + 1) * FH],
                                    start=(g == 0), stop=(g == NG - 1))
                            nc.vector.tensor_copy(
                                v_sb[tpair][:, jpl, half * FH:(half + 1) * FH], ps[:])
                            if AV_FP8:
                                nc.vector.tensor_copy(
                                    v8_sb[tpair][:, jpl, half * FH:(half + 1) * FH], ps[:])
                # cv column sums (bf16 v): ones stationary, single accumulation
                # group per PSUM region (columns on the free dim)
                cva = psa.tile([P, QC], f32, tag="cva", bufs=1)
                cvb = psa.tile([P, D - QC], f32, tag="cvb", bufs=1)
                for kt in range(KT):
                    tpair, jpl = divmod(kt, 2)
                    nc.tensor.matmul(cva[:], ones128[:], v_sb[tpair][:, jpl, 0:QC],
                                     start=(kt == 0), stop=(kt == KT - 1))
                    nc.tensor.matmul(cvb[:], ones128[:], v_sb[tpair][:, jpl, QC:D],
                                     start=(kt == 0), stop=(kt == KT - 1))
                cv_row = pa.tile([1, D], f32, tag="cv_row")
                nc.vector.tensor_copy(cv_row[0:1, 0:QC], cva[0:1, :])
                nc.vector.tensor_copy(cv_row[0:1, QC:D], cvb[0:1, :])
                for e in range(NG):
                    nc.gpsimd.dma_start(cv_sb[:, e:e + 1],
                                        cv_row[0:1, e * P:(e + 1) * P])

            # ---- Phase B: qT ----
            with tc.tile_pool(name="pB", bufs=1) as pb, \
                 tc.tile_pool(name="psB", bufs=1, space="PSUM") as psb:
                for c in range(NQC):
                    if c == 0:
                        xq_sb = xq0
                    else:
                        xq_sb = pb.tile([P, NG, QC], bf16, tag="xq", bufs=3, name="xq")
                        nc.sync.dma_start(xq_sb[:], xqT[:, :, c * QC:(c + 1) * QC])
                    for ft in range(NG):
                        ps = psb.tile([P, QC], f32, tag="psq", bufs=2)
                        for g in range(NG):
                            nc.tensor.matmul(ps[:], wq_sb[:, g, ft * P:(ft + 1) * P],
                                             xq_sb[:, g, :],
                                             start=(g == 0), stop=(g == NG - 1))
                        nc.vector.tensor_scalar_add(
                            qT_sb[:, ft, c * QC:(c + 1) * QC], ps[:],
                            bq_sb[:, ft:ft + 1])

            # ---- Phase C: attention + RS + norm + scatter ----
            with tc.tile_pool(name="pC", bufs=2) as pc, \
                 tc.tile_pool(name="pE", bufs=2) as pe, \
                 tc.tile_pool(name="psC", bufs=1, space="PSUM") as psc:

                def qk(kt, qc, aps):
                    if QK_FP8:
                        for a in range(3):
                            nc.tensor.matmul(
                                aps[:], kT_sb[:, 2 * a:2 * a + 2, kt * P:(kt + 1) * P],
                                qT_sb[:, 2 * a:2 * a + 2, qc * QC:(qc + 1) * QC],
                                start=(a == 0), stop=(a == 2), perf_mode=DR)
                    else:
                        for g in range(NG):
                            nc.tensor.matmul(
                                aps[:], kT_sb[:, g, kt * P:(kt + 1) * P],
                                qT_sb[:, g, qc * QC:(qc + 1) * QC],
                                start=(g == 0), stop=(g == NG - 1))

                def av1(t, e_tile, ypsum, e):
                    if AV_FP8:
                        nc.tensor.matmul(
                            ypsum[e][:], v8_sb[t][:, :, e * P:(e + 1) * P],
                            e_tile[:], start=(t == 0), stop=(t == KTP - 1),
                            perf_mode=DR)
                    else:
                        for j in range(2):
                            nc.tensor.matmul(
                                ypsum[e][:], v8_sb[t][:, j, e * P:(e + 1) * P],
                                e_tile[:, j, :],
                                start=(t == 0 and j == 0),
                                stop=(t == KTP - 1 and j == 1))

                def norm_scatter(b):
                    # normalize RS-completed block b and scatter into fT3
                    s_row = pe.tile([1, QC], bf16, tag="s_row", name="s_row")
                    nc.sync.dma_start(s_row[:], rs_out[b][FH:FH + 1, :])
                    s32 = pe.tile([1, QC], f32, tag="s32", name="s32")
                    nc.vector.tensor_copy(s32[:], s_row[:])
                    r32 = pe.tile([1, QC], f32, tag="r32", name="r32")
                    nc.vector.reciprocal_approx_fast(r32[:], s32[:])
                    rec = pe.tile([P, QC], f32, tag="rec", name="rec")
                    nc.gpsimd.partition_broadcast(rec[:], r32[:])
                    fT3r = fT3[:].rearrange("p g (v sg) -> p g v sg", sg=16)
                    for m in range(3):
                        fr = pe.tile([P, QC], bf16, tag="fr", bufs=2, name="fr")
                        nc.sync.dma_start(fr[:], rs_out[b][m * P:(m + 1) * P, :])
                        fn = pe.tile([P, QC], bf16, tag="fn", bufs=2, name="fn")
                        nc.vector.tensor_mul(fn[:], fr[:], rec[:])
                        for (j0, na, gi0, sseg) in segs[(b, m)]:
                            tmp = pe.tile([P, 4, P], bf16, tag="scat", bufs=2,
                                          name="scat")
                            nc.sync.dma_start(tmp[:, 0:na, :],
                                              fn[:, j0:j0 + na * P],
                                              transpose=True)
                            nc.vector.tensor_copy(
                                fT3r[:, gi0:gi0 + na, :, sseg], tmp[:, 0:na, :])

                for qc in range(NQC):
                    ypsum = [psc.tile([P, QC], f32, name=f"y{e}", tag=f"y{e}", bufs=1)
                             for e in range(NG)]
                    sums_ps = psc.tile([8, QC], f32, tag="sums", bufs=1)
                    pairs = {}

                    def sums_mm(kt):
                        t, j = divmod(kt, 2)
                        nc.tensor.matmul(sums_ps[:], ones8[:], pairs[t][0][:, j, :],
                                         start=(kt == 0), stop=(kt == KT - 1),
                                         skip_group_check=True)

                    for t in range(KTP):
                        a_pair = pc.tile([P, 2, QC], bf16, tag="a_pair", bufs=3)
                        e_tile = pc.tile([P, 2, QC], av_dt, tag="e_tile", bufs=3)
                        pairs[t] = (a_pair, e_tile)
                        for j in range(2):
                            kt = 2 * t + j
                            aps = psc.tile([P, QC], f32, tag="att", bufs=1)
                            qk(kt, qc, aps)
                            nc.scalar.activation(a_pair[:, j, :], aps[:], Exp,
                                                 scale=SCALE)
                            nc.scalar.activation(e_tile[:, j, :], a_pair[:, j, :],
                                                 Identity, bias=neg1[:])
                            # fill PE pipeline behind this QK with prev-pair work
                            # so the single aps buffer never stalls the PE
                            if t > 0:
                                sums_mm(2 * (t - 1) + j)
                                for e in (range(3) if j == 0 else range(3, NG)):
                                    av1(t - 1, pairs[t - 1][1], ypsum, e)
                    for j in range(2):
                        sums_mm(2 * (KTP - 1) + j)
                        for e in (range(3) if j == 0 else range(3, NG)):
                            av1(KTP - 1, pairs[KTP - 1][1], ypsum, e)

                    # epilogue: sums row + unnormalized yTaug writes + RS
                    sums_bf = pc.tile([1, QC], bf16, tag="sums_bf")
                    nc.vector.tensor_copy(sums_bf[:], sums_ps[0:1, :])
                    yb = yTaug[qc]
                    nc.sync.dma_start(yb[FH:FH + 1, :], sums_bf[:])
                    nc.sync.dma_start(yb[2 * FH + 1:2 * FH + 2, :], sums_bf[:])
                    for e in range(NG):
                        yt = pc.tile([P, QC], bf16, tag="yt", bufs=3)
                        nc.vector.tensor_scalar_add(yt[:], ypsum[e][:],
                                                    cv_sb[:, e:e + 1])
                        half, m = divmod(e, 3)
                        nc.sync.dma_start(
                            yb[(FH + 1) * half + m * P:(FH + 1) * half + m * P + P, :],
                            yt[:])
                    nc.gpsimd.collective_compute(
                        "ReduceScatter", mybir.AluOpType.add,
                        replica_groups=GROUPS,
                        ins=[yTaug[qc].opt()], outs=[rs_out[qc].opt()])
                    if qc > 0:
                        norm_scatter(qc - 1)
                norm_scatter(NQC - 1)

            # ---- Phase F: out = y_perm @ Wc.T + bc ----
            with tc.tile_pool(name="pF", bufs=1) as pf, \
                 tc.tile_pool(name="psF", bufs=2, space="PSUM") as psf:
                for t in range(SK // P):
                    bvt = pf.tile([P, D], f32, tag="bvt", bufs=3, name="bvt")
                    nc.scalar.dma_start(bvt[:], bvout[t * P:(t + 1) * P, :])
                    po = psf.tile([P, QC], f32, tag="po")
                    po2 = psf.tile([P, D - QC], f32, tag="po2")
                    for g in range(NG):
                        lhsT = fT3[:, g, t * P:(t + 1) * P]
                        nc.tensor.matmul(po[:], lhsT, wc_sb[:, g, 0:QC],
                                         start=(g == 0), stop=(g == NG - 1))
                        nc.tensor.matmul(po2[:], lhsT, wc_sb[:, g, QC:D],
                                         start=(g == 0), stop=(g == NG - 1))
                    o_sb = pf.tile([P, D], f32, tag="o_sb", bufs=3)
                    nc.vector.tensor_add(o_sb[:, 0:QC], po[:], bvt[:, 0:QC])
                    nc.vector.tensor_add(o_sb[:, QC:D], po2[:], bvt[:, QC:D])
                    nc.sync.dma_start(out[t * P:(t + 1) * P, :], o_sb[:])

    return nc


def _get_nc():
    global _nc
    if _nc is None:
        _nc = _build_program()
        _nc.finalize()
    return _nc


def _gmaj(w):
    # [D, D] (row f_out, col d) -> [P, NG, D]: [p, g, f] = w[f, 128g + p]
    return np.ascontiguousarray(
        w.T.reshape(NG, P, D).transpose(1, 0, 2)).astype(BF16)


def _prep_in_maps(x, Wq, bq, Wk, bk, Wv, bv, Wc, bc):
    x = np.asarray(x, dtype=np.float32)
    Wq = np.asarray(Wq, np.float32); Wk = np.asarray(Wk, np.float32)
    Wv = np.asarray(Wv, np.float32); Wc = np.asarray(Wc, np.float32)
    bqf = np.asarray(bq, np.float32); bkf = np.asarray(bk, np.float32)
    bvf = np.asarray(bv, np.float32); bcf = np.asarray(bc, np.float32)

    phi = np.array([_phi(f) for f in range(D)])
    wq4 = _gmaj(Wq)
    wk4 = _gmaj(Wk)
    wv4 = _gmaj(Wv[phi])          # permuted output columns
    wc4 = _gmaj(Wc)               # wc_sb[p, g, e] = Wc[e, 128g+p]
    bq_c = np.ascontiguousarray(bqf.reshape(NG, P).T)
    bk_c = np.ascontiguousarray(bkf.reshape(NG, P).T)

    # bvout[s, e] = sum_d bv[f(s, d)] * Wc[e, d] + bc[e], per half h:
    # the +bv term of the normalized y, pushed through the permutation and
    # the output projection on the host.
    bvouts = []
    for h in range(2):
        flat = 768 * (SK * h) + np.arange(SK * D)
        ybv = bvf[flat // S].reshape(SK, D)      # y_perm rows of the bv field
        bvouts.append((ybv @ Wc.T + bcf).astype(np.float32))

    in_maps = []
    for c in range(8):
        b, h = divmod(c, 2)
        xT = x[b].T.astype(BF16)                      # [D, S]
        xq4 = np.ascontiguousarray(xT.reshape(NG, P, S).transpose(1, 0, 2))
        xkv4 = np.ascontiguousarray(
            xT[:, h * SK:(h + 1) * SK].reshape(NG, P, SK).transpose(1, 0, 2))
        in_maps.append({
            "xqT": xq4, "xkvT": xkv4,
            "wqT": wq4, "wkT": wk4, "wvT": wv4, "wcT": wc4,
            "bq_c": bq_c, "bk_c": bk_c, "bvout": bvouts[h],
        })
    return in_maps


def _assemble(results):
    out = np.empty((B, S, D), dtype=np.float32)
    for c in range(8):
        b, h = divmod(c, 2)
        out[b, h * SK:(h + 1) * SK, :] = results[c]["out"]
    return out


def run_on_hw(trace=False, **inputs):
    from concourse.bass_utils import run_bass_kernel_spmd
    nc = _get_nc()
    in_maps = _prep_in_maps(**inputs)
    res = run_bass_kernel_spmd(nc, in_maps, list(range(8)), trace=trace)
    return _assemble(res.results), res


def kernel(**inputs):
    out, _ = run_on_hw(trace=False, **inputs)
    return out


# revision 32
# speedup vs baseline: 1.2462x; 1.0634x over previous
"""Trainium2 Bass kernel for single-head full-dim attention (nn_CasualSelfAttention).

Reference math (B=4, S=4096, D=768, fp32):
    q = x @ Wq.T + bq ; k = x @ Wk.T + bk ; v = x @ Wv.T + bv
    att = softmax(q @ k.T * D**-0.5)        # no mask
    y = att @ v
    y = y.transpose(0,2,1).reshape(B,S,D)   # element permutation
    out = y @ Wc.T + bc

Sharding (8 cores): core c = 2*b + h handles batch b, all 4096 queries, its
half of the keys (rows h*2048:(h+1)*2048). Pairwise ReduceScatter (bf16) by
feature rows hands core h the reduced feature slice [384h:384h+384] for all
queries == exactly output rows [2048h:2048h+2048] after the permutation.

v2 numerics / structure:
  - host pre-transposes x and weights (no device transpose DMAs on the way in)
  - exp(z) = 1 + E decomposition: AV matmul runs on the residual E in fp8e4
    (DoubleRow, 2x PE rate) against fp8 v, while the "1"-weighted part is the
    exact bf16 column-sum of v (cv), added in the epilogue. QK^T also runs
    fp8 DoubleRow on fp8 q,k. Projections stay bf16 (precision budget).
  - the value bias bv is applied after normalization (y/s + bv), so v is
    projected without bias and sums need no folding.
  - v's columns are permuted (phi) so the y^T partial rows land in yTaug as
    3 "m-planes" (x = 3u + m), which makes the post-RS permutation scatter a
    set of 4 rectangular 128-partition XBAR transpose DMAs per RS block into
    a sigma-major SBUF buffer fT (col = (s%16)*128 + s//16). Phase F reads fT
    through a 3D weight AP and un-scrambles rows in the output DMA. No DRAM
    roundtrip, no serial transpose tail.
  - sums: DVE accumulates exp tiles pairwise, GpSimd partition_all_reduce
    does the 128->1 key-partition reduction, normalization uses
    broadcast + vector reciprocal.
"""

import numpy as np
import ml_dtypes

BF16 = ml_dtypes.bfloat16
F8 = ml_dtypes.float8_e4m3fn

B, S, D = 4, 4096, 768
SK = S // 2            # keys per core
P = 128
NG = D // P            # 6 feature groups of 128
QC = 512               # query chunk width == RS block width
NQC = S // QC          # 8 query chunks / RS blocks
KT = SK // P           # 16 key tiles
KTP = KT // 2          # 8 key-tile pairs
FH = D // 2            # 384 features per half (RS slice)
SCALE = float(D) ** -0.5
GROUPS = [[0, 1], [2, 3], [4, 5], [6, 7]]

QK_FP8 = True
AV_FP8 = True

_nc = None


def _phi(fp):
    """v-column permutation: ypsum[e] partition p holds feature phi(128e+p)."""
    e, mcol = divmod(fp, P)
    return 384 * (e // 3) + 3 * mcol + (e % 3)


def _scatter_segments():
    """Per (block b, m-plane m): list of (j0, na, gi0, sseg) transpose calls.

    dst[p, a, u] = fn_m[u, j0 + 128a + p] lands at fT3[:, gi0+a, sseg + 16u]
    (natural s columns, stride-16 dst).
    """
    out = {}
    for b in range(NQC):
        c0 = QC * b
        for m in range(3):
            base = m * S + c0
            d0, s0 = base % D, base // D
            segs = []
            jw = D - d0
            if jw >= QC:
                segs.append((0, QC, d0, s0))
            else:
                segs.append((0, jw, d0, s0))
                segs.append((jw, QC - jw, 0, s0 + 1))
            calls = []
            for (j0, jl, dseg, sseg) in segs:
                assert jl % P == 0 and dseg % P == 0 and sseg < 16
                calls.append((j0, jl // P, dseg // P, sseg))
            out[(b, m)] = calls
    return out


def _build_program():
    import concourse.bass as bass
    import concourse.mybir as mybir
    import concourse.tile as tile
    from concourse import bacc
    from concourse import bass_isa

    f32 = mybir.dt.float32
    bf16 = mybir.dt.bfloat16
    fp8 = mybir.dt.float8e4
    Exp = mybir.ActivationFunctionType.Exp
    Identity = mybir.ActivationFunctionType.Identity
    mult = mybir.AluOpType.mult
    DR = mybir.MatmulPerfMode.DoubleRow

    qk_dt = fp8 if QK_FP8 else bf16
    av_dt = fp8 if AV_FP8 else bf16
    segs = _scatter_segments()

    nc = bacc.Bacc(None, num_devices=8)

    xqT = nc.declare_dram_parameter("xqT", [NQC, P, NG, QC], bf16, isOutput=False)
    xkvT = nc.declare_dram_parameter("xkvT", [SK // QC, P, NG, QC], bf16, isOutput=False)
    wqT = nc.declare_dram_parameter("wqT", [P, NG, D], bf16, isOutput=False)
    wkT = nc.declare_dram_parameter("wkT", [P, NG, D], bf16, isOutput=False)
    wvT = nc.declare_dram_parameter("wvT", [P, NG, D], bf16, isOutput=False)
    wcT = nc.declare_dram_parameter("wcT", [P, NG, D], bf16, isOutput=False)
    bq_c = nc.declare_dram_parameter("bq_c", [P, NG], f32, isOutput=False)
    bk_c = nc.declare_dram_parameter("bk_c", [P, NG], f32, isOutput=False)
    bvout = nc.declare_dram_parameter("bvout", [SK, D], f32, isOutput=False)
    out = nc.declare_dram_parameter("out", [SK, D], f32, isOutput=True)

    with tile.TileContext(nc) as tc:
        with tc.tile_pool(name="persist", bufs=1) as pp, \
             tc.tile_pool(name="dram", bufs=1, space="DRAM") as dram:
            yTaug = [dram.tile([2 * (FH + 1), QC], bf16, name=f"yTaug{b}", tag=f"yTaug{b}")
                     for b in range(NQC)]
            rs_out = [dram.tile([FH + 1, QC], bf16, name=f"rs_out{b}", tag=f"rs_out{b}")
                      for b in range(NQC)]

            # ---- persistent SBUF ----
            kT_sb = pp.tile([P, NG, SK], qk_dt, tag="kT")
            qT_sb = pp.tile([P, NG, S], qk_dt, tag="qT")
            v_sb = [pp.tile([P, 2, D], bf16, name=f"v{t}", tag=f"v{t}") for t in range(KTP)]
            v8_sb = [pp.tile([P, 2, D], av_dt, name=f"v8{t}", tag=f"v8{t}") for t in range(KTP)] \
                if AV_FP8 else v_sb
            fT3 = pp.tile([P, NG, SK], bf16, tag="fT3")
            wc_sb = pp.tile([P, NG, D], bf16, tag="wc_sb")
            bq_sb = pp.tile([P, NG], f32, tag="bq_sb")
            bk_sb = pp.tile([P, NG], f32, tag="bk_sb")
            cv_sb = pp.tile([P, NG], f32, tag="cv_sb")
            ones1 = pp.tile([P, 1], bf16, tag="ones1")
            xq0 = pp.tile([P, NG, QC], bf16, tag="xq0")

            wq_sb = pp.tile([P, NG, D], bf16, tag="wq_sb")
            ones8 = pp.tile([P, 8], bf16, tag="ones8")
            neg1 = pp.tile([P, 1], f32, tag="neg1")
            nc.vector.memset(ones1[:], 1.0)
            nc.vector.memset(ones8[:], 1.0)
            nc.vector.memset(neg1[:], -1.0)
            nc.scalar.dma_start(bq_sb[:], bq_c[:])
            nc.scalar.dma_start(bk_sb[:], bk_c[:])
            nc.scalar.dma_start(wq_sb[:], wqT[:])
            nc.scalar.dma_start(xq0[:], xqT[0])
            nc.scalar.dma_start(wc_sb[:], wcT[:])

            # ---- Phase A: kT (fp8/bf16), v (bf16 + fp8), cv ----
            with tc.tile_pool(name="pA", bufs=1) as pa, \
                 tc.tile_pool(name="psA", bufs=1, space="PSUM") as psa:
                wk_sb = pa.tile([P, NG, D], bf16, tag="wk_sb")
                wv_sb = pa.tile([P, NG, D], bf16, tag="wv_sb")
                nc.sync.dma_start(wk_sb[:], wkT[:])
                ones128 = pa.tile([P, P], bf16, tag="ones128")
                nc.vector.memset(ones128[:], 1.0)
                wv_loaded = False
                for kc in range(SK // QC):
                    xkv_sb = pa.tile([P, NG, QC], bf16, tag="xkv", bufs=3, name="xkv")
                    nc.sync.dma_start(xkv_sb[:], xkvT[kc])
                    if not wv_loaded:
                        nc.sync.dma_start(wv_sb[:], wvT[:])
                        wv_loaded = True
                    # k projection -> kT_sb
                    for ft in range(NG):
                        ps = psa.tile([P, QC], f32, tag="psk", bufs=2)
                        for g in range(NG):
                            nc.tensor.matmul(ps[:], wk_sb[:, g, ft * P:(ft + 1) * P],
                                             xkv_sb[:, g, :],
                                             start=(g == 0), stop=(g == NG - 1))
                        nc.vector.tensor_scalar_add(
                            kT_sb[:, ft, kc * QC:(kc + 1) * QC], ps[:],
                            bk_sb[:, ft:ft + 1])
                    # v projection (no bias; phi-permuted columns via wvT)
                    for tl in range(QC // P):
                        kt = kc * (QC // P) + tl
                        tpair, jpl = divmod(kt, 2)
                        for half in range(2):
                            ps = psa.tile([P, FH], f32, tag="psv", bufs=2)
                            for g in range(NG):
                                nc.tensor.matmul(
                                    ps[:], xkv_sb[:, g, tl * P:(tl + 1) * P],
                                    wv_sb[:, g, half * FH:(half + 1) * FH],
                                    start=(g == 0), stop=(g == NG - 1))
                            nc.vector.tensor_copy(
                                v_sb[tpair][:, jpl, half * FH:(half + 1) * FH], ps[:])
                            if AV_FP8:
                                nc.vector.tensor_copy(
                                    v8_sb[tpair][:, jpl, half * FH:(half + 1) * FH], ps[:])
                # cv column sums (bf16 v): ones stationary, single accumulation
                # group per PSUM region (columns on the free dim)
                cva = psa.tile([P, QC], f32, tag="cva", bufs=1)
                cvb = psa.tile([P, D - QC], f32, tag="cvb", bufs=1)
                for kt in range(KT):
                    tpair, jpl = divmod(kt, 2)
                    nc.tensor.matmul(cva[:], ones128[:], v_sb[tpair][:, jpl, 0:QC],
                                     start=(kt == 0), stop=(kt == KT - 1))
                    nc.tensor.matmul(cvb[:], ones128[:], v_sb[tpair][:, jpl, QC:D],
                                     start=(kt == 0), stop=(kt == KT - 1))
                cv_row = pa.tile([1, D], f32, tag="cv_row")
                nc.vector.tensor_copy(cv_row[0:1, 0:QC], cva[0:1, :])
                nc.vector.tensor_copy(cv_row[0:1, QC:D], cvb[0:1, :])
                for e in range(NG):
                    nc.gpsimd.dma_start(cv_sb[:, e:e + 1],
                                        cv_row[0:1, e * P:(e + 1) * P])

            # ---- Phase B: qT ----
            with tc.tile_pool(name="pB", bufs=1) as pb, \
                 tc.tile_pool(name="psB", bufs=1, space="PSUM") as psb:
                for c in range(NQC):
                    if c == 0:
                        xq_sb = xq0
                    else:
                        xq_sb = pb.tile([P, NG, QC], bf16, tag="xq", bufs=3, name="xq")
                        nc.sync.dma_start(xq_sb[:], xqT[c])
                    for ft in range(NG):
                        ps = psb.tile([P, QC], f32, tag="psq", bufs=2)
                        for g in range(NG):
                            nc.tensor.matmul(ps[:], wq_sb[:, g, ft * P:(ft + 1) * P],
                                             xq_sb[:, g, :],
                                             start=(g == 0), stop=(g == NG - 1))
                        nc.vector.tensor_scalar_add(
                            qT_sb[:, ft, c * QC:(c + 1) * QC], ps[:],
                            bq_sb[:, ft:ft + 1])

            # ---- Phase C: attention + RS + norm + scatter ----
            with tc.tile_pool(name="pC", bufs=2) as pc, \
                 tc.tile_pool(name="pE", bufs=2) as pe, \
                 tc.tile_pool(name="psC", bufs=1, space="PSUM") as psc:

                def qk(kt, qc, aps):
                    if QK_FP8:
                        for a in range(3):
                            nc.tensor.matmul(
                                aps[:], kT_sb[:, 2 * a:2 * a + 2, kt * P:(kt + 1) * P],
                                qT_sb[:, 2 * a:2 * a + 2, qc * QC:(qc + 1) * QC],
                                start=(a == 0), stop=(a == 2), perf_mode=DR)
                    else:
                        for g in range(NG):
                            nc.tensor.matmul(
                                aps[:], kT_sb[:, g, kt * P:(kt + 1) * P],
                                qT_sb[:, g, qc * QC:(qc + 1) * QC],
                                start=(g == 0), stop=(g == NG - 1))

                def av1(t, e_tile, ypsum, e):
                    if AV_FP8:
                        nc.tensor.matmul(
                            ypsum[e][:], v8_sb[t][:, :, e * P:(e + 1) * P],
                            e_tile[:], start=(t == 0), stop=(t == KTP - 1),
                            perf_mode=DR)
                    else:
                        for j in range(2):
                            nc.tensor.matmul(
                                ypsum[e][:], v8_sb[t][:, j, e * P:(e + 1) * P],
                                e_tile[:, j, :],
                                start=(t == 0 and j == 0),
                                stop=(t == KTP - 1 and j == 1))

                def norm_scatter(b):
                    # normalize RS-completed block b and scatter into fT3
                    s_row = pe.tile([1, QC], bf16, tag="s_row", name="s_row")
                    nc.sync.dma_start(s_row[:], rs_out[b][FH:FH + 1, :])
                    s32 = pe.tile([1, QC], f32, tag="s32", name="s32")
                    nc.vector.tensor_copy(s32[:], s_row[:])
                    r32 = pe.tile([1, QC], f32, tag="r32", name="r32")
                    nc.vector.reciprocal_approx_fast(r32[:], s32[:])
                    rec = pe.tile([P, QC], f32, tag="rec", name="rec")
                    nc.gpsimd.partition_broadcast(rec[:], r32[:])
                    fT3r = fT3[:].rearrange("p g (v sg) -> p g v sg", sg=16)
                    for m in range(3):
                        fr = pe.tile([P, QC], bf16, tag="fr", bufs=2, name="fr")
                        nc.sync.dma_start(fr[:], rs_out[b][m * P:(m + 1) * P, :])
                        fn = pe.tile([P, QC], bf16, tag="fn", bufs=2, name="fn")
                        nc.vector.tensor_mul(fn[:], fr[:], rec[:])
                        for (j0, na, gi0, sseg) in segs[(b, m)]:
                            tmp = pe.tile([P, 4, P], bf16, tag="scat", bufs=2,
                                          name="scat")
                            nc.sync.dma_start(tmp[:, 0:na, :],
                                              fn[:, j0:j0 + na * P],
                                              transpose=True)
                            nc.vector.tensor_copy(
                                fT3r[:, gi0:gi0 + na, :, sseg], tmp[:, 0:na, :])

                for qc in range(NQC):
                    ypsum = [psc.tile([P, QC], f32, name=f"y{e}", tag=f"y{e}", bufs=1)
                             for e in range(NG)]
                    sums_ps = psc.tile([8, QC], f32, tag="sums", bufs=1)
                    pairs = {}

                    def sums_mm(kt):
                        t, j = divmod(kt, 2)
                        nc.tensor.matmul(sums_ps[:], ones8[:], pairs[t][0][:, j, :],
                                         start=(kt == 0), stop=(kt == KT - 1),
                                         skip_group_check=True)

                    for t in range(KTP):
                        a_pair = pc.tile([P, 2, QC], bf16, tag="a_pair", bufs=3)
                        e_tile = pc.tile([P, 2, QC], av_dt, tag="e_tile", bufs=3)
                        pairs[t] = (a_pair, e_tile)
                        for j in range(2):
                            kt = 2 * t + j
                            aps = psc.tile([P, QC], f32, tag="att", bufs=1)
                            qk(kt, qc, aps)
                            nc.scalar.activation(a_pair[:, j, :], aps[:], Exp,
                                                 scale=SCALE)
                            nc.scalar.activation(e_tile[:, j, :], a_pair[:, j, :],
                                                 Identity, bias=neg1[:])
                            # fill PE pipeline behind this QK with prev-pair work
                            # so the single aps buffer never stalls the PE
                            if t > 0:
                                sums_mm(2 * (t - 1) + j)
                                for e in (range(3) if j == 0 else range(3, NG)):
                                    av1(t - 1, pairs[t - 1][1], ypsum, e)
                    for j in range(2):
                        sums_mm(2 * (KTP - 1) + j)
                        for e in (range(3) if j == 0 else range(3, NG)):
                            av1(KTP - 1, pairs[KTP - 1][1], ypsum, e)

                    # epilogue: sums row + unnormalized yTaug writes + RS
                    sums_bf = pc.tile([1, QC], bf16, tag="sums_bf")
                    nc.vector.tensor_copy(sums_bf[:], sums_ps[0:1, :])
                    yb = yTaug[qc]
                    nc.sync.dma_start(yb[FH:FH + 1, :], sums_bf[:])
                    nc.sync.dma_start(yb[2 * FH + 1:2 * FH + 2, :], sums_bf[:])
                    for e in range(NG):
                        yt = pc.tile([P, QC], bf16, tag="yt", bufs=6)
                        if e < 3:
                            nc.scalar.activation(yt[:], ypsum[e][:], Identity,
                                                 bias=cv_sb[:, e:e + 1])
                        else:
                            nc.vector.tensor_scalar_add(yt[:], ypsum[e][:],
                                                        cv_sb[:, e:e + 1])
                        half, m = divmod(e, 3)
                        nc.sync.dma_start(
                            yb[(FH + 1) * half + m * P:(FH + 1) * half + m * P + P, :],
                            yt[:])
                    nc.gpsimd.collective_compute(
                        "ReduceScatter", mybir.AluOpType.add,
                        replica_groups=GROUPS,
                        ins=[yTaug[qc].opt()], outs=[rs_out[qc].opt()])
                    if qc > 0:
                        norm_scatter(qc - 1)
                norm_scatter(NQC - 1)

                # ---- Phase F (same pool scope: no transition barrier) ----
                for t in range(SK // P):
                    bvt = pe.tile([P, D], f32, tag="bvt", bufs=3, name="bvt")
                    nc.scalar.dma_start(bvt[:], bvout[t * P:(t + 1) * P, :])
                    po = psc.tile([P, QC], f32, tag=f"y{2 * (t % 2)}", bufs=1)
                    po2 = psc.tile([P, QC], f32, tag=f"y{2 * (t % 2) + 1}", bufs=1)
                    for g in range(NG):
                        lhsT = fT3[:, g, t * P:(t + 1) * P]
                        nc.tensor.matmul(po[:], lhsT, wc_sb[:, g, 0:QC],
                                         start=(g == 0), stop=(g == NG - 1))
                        nc.tensor.matmul(po2[:, 0:D - QC], lhsT, wc_sb[:, g, QC:D],
                                         start=(g == 0), stop=(g == NG - 1))
                    o_sb = pe.tile([P, D], f32, tag="o_sb", bufs=3)
                    nc.vector.tensor_add(o_sb[:, 0:QC], po[:], bvt[:, 0:QC])
                    nc.vector.tensor_add(o_sb[:, QC:D], po2[:, 0:D - QC],
                                         bvt[:, QC:D])
                    nc.sync.dma_start(out[t * P:(t + 1) * P, :], o_sb[:])

    return nc


def _get_nc():
    global _nc
    if _nc is None:
        _nc = _build_program()
        _nc.finalize()
    return _nc


def _gmaj(w):
    # [D, D] (row f_out, col d) -> [P, NG, D]: [p, g, f] = w[f, 128g + p]
    return np.ascontiguousarray(
        w.T.reshape(NG, P, D).transpose(1, 0, 2)).astype(BF16)


def _prep_in_maps(x, Wq, bq, Wk, bk, Wv, bv, Wc, bc):
    x = np.asarray(x, dtype=np.float32)
    Wq = np.asarray(Wq, np.float32); Wk = np.asarray(Wk, np.float32)
    Wv = np.asarray(Wv, np.float32); Wc = np.asarray(Wc, np.float32)
    bqf = np.asarray(bq, np.float32); bkf = np.asarray(bk, np.float32)
    bvf = np.asarray(bv, np.float32); bcf = np.asarray(bc, np.float32)

    phi = np.array([_phi(f) for f in range(D)])
    wq4 = _gmaj(Wq)
    wk4 = _gmaj(Wk)
    wv4 = _gmaj(Wv[phi])          # permuted output columns
    wc4 = _gmaj(Wc)               # wc_sb[p, g, e] = Wc[e, 128g+p]
    bq_c = np.ascontiguousarray(bqf.reshape(NG, P).T)
    bk_c = np.ascontiguousarray(bkf.reshape(NG, P).T)

    # bvout[s, e] = sum_d bv[f(s, d)] * Wc[e, d] + bc[e], per half h:
    # the +bv term of the normalized y, pushed through the permutation and
    # the output projection on the host.
    bvouts = []
    for h in range(2):
        flat = 768 * (SK * h) + np.arange(SK * D)
        ybv = bvf[flat // S].reshape(SK, D)      # y_perm rows of the bv field
        bvouts.append((ybv @ Wc.T + bcf).astype(np.float32))

    in_maps = []
    for c in range(8):
        b, h = divmod(c, 2)
        xT = x[b].T.astype(BF16)                      # [D, S]
        # chunk-major: [chunk, p, g, col] so chunk loads are contiguous
        xq4 = np.ascontiguousarray(
            xT.reshape(NG, P, NQC, QC).transpose(2, 1, 0, 3))
        xkv4 = np.ascontiguousarray(
            xT[:, h * SK:(h + 1) * SK].reshape(NG, P, SK // QC, QC)
            .transpose(2, 1, 0, 3))
        in_maps.append({
            "xqT": xq4, "xkvT": xkv4,
            "wqT": wq4, "wkT": wk4, "wvT": wv4, "wcT": wc4,
            "bq_c": bq_c, "bk_c": bk_c, "bvout": bvouts[h],
        })
    return in_maps


def _assemble(results):
    out = np.empty((B, S, D), dtype=np.float32)
    for c in range(8):
        b, h = divmod(c, 2)
        out[b, h * SK:(h + 1) * SK, :] = results[c]["out"]
    return out


def run_on_hw(trace=False, **inputs):
    from concourse.bass_utils import run_bass_kernel_spmd
    nc = _get_nc()
    in_maps = _prep_in_maps(**inputs)
    res = run_bass_kernel_spmd(nc, in_maps, list(range(8)), trace=trace)
    return _assemble(res.results), res


def kernel(**inputs):
    out, _ = run_on_hw(trace=False, **inputs)
    return out
